# revision 9
# baseline (speedup 1.0000x reference)
"""Trainium2 Bass kernel for the MoE problem (top-2-of-8 routing + shared expert).

Strategy (expert-parallel over 8 NeuronCores):
  - Core c owns expert c (dense over all T=4096 tokens: with E=8, K=2 the
    dense per-expert compute spread over 8 cores equals the total routed
    FLOPs, with zero load imbalance and no token dispatch) and a 1/8 slice
    of the shared expert's intermediate dimension.
  - The router (logits -> top-2 -> softmax combine weights) is computed
    replicated on every core in exact fp32; each core extracts its expert's
    per-token combine weight with a one-hot mask input.
  - Main GEMMs run in float32r (reduced-precision fp32 matmul mode, ~4x the
    fp32 rate on the PE array).
  - Per 512-token chunk, each core produces its partial output in
    transposed [H, Tc] layout (expert output scaled by combine weight plus
    shared-expert partial); a ReduceScatter over the 8 cores sums the
    partials, leaving each core with a 128-row H-slice. The host stitches
    the slices and transposes back.

kernel(**inputs) takes the full unsharded inputs and returns the full
output, matching reference.reference()'s (out, router_z_loss) structure.
"""

import numpy as np
from contextlib import ExitStack

import concourse.bass as bass
import concourse.mybir as mybir
import concourse.tile as tile
from concourse import bacc
from concourse.bass_utils import run_bass_kernel_spmd

F32 = mybir.dt.float32
F32R = mybir.dt.float32r
AF = mybir.ActivationFunctionType
ALU = mybir.AluOpType
AX = mybir.AxisListType

# Problem shape (hardcoded per the harness contract).
B, S, H, E, I, IS = 2, 2048, 1024, 8, 2048, 4096
T = B * S                  # 4096 tokens
NCORES = 8
ISH = IS // NCORES         # 512 shared-intermediate per core
TC = 512                   # token chunk
NCH = T // TC              # 8 chunks
KH = H // 128              # 8
KI = I // 128              # 16
KS = ISH // 128            # 4
HT = H // 128              # 8 output H tiles
NT4 = TC // 128            # 4 token tiles per chunk


def _build(nc):
    xw = nc.dram_tensor("xw", [128, KH, T], F32R, kind="ExternalInput").ap()
    rt = nc.dram_tensor("rt", [128, KH, E], F32, kind="ExternalInput").ap()
    em = nc.dram_tensor("em", [128, E], F32, kind="ExternalInput").ap()
    wg = nc.dram_tensor("wg", [128, KI, KH, 128], F32R, kind="ExternalInput").ap()
    wu = nc.dram_tensor("wu", [128, KI, KH, 128], F32R, kind="ExternalInput").ap()
    wd = nc.dram_tensor("wd", [128, HT, KI, 128], F32R, kind="ExternalInput").ap()
    sg = nc.dram_tensor("sg", [128, KS, KH, 128], F32R, kind="ExternalInput").ap()
    su = nc.dram_tensor("su", [128, KS, KH, 128], F32R, kind="ExternalInput").ap()
    sd = nc.dram_tensor("sd", [128, HT, KS, 128], F32R, kind="ExternalInput").ap()
    yout = nc.dram_tensor("y", [NCH, 2, 64, TC], F32, kind="ExternalOutput").ap()

    with tile.TileContext(nc) as tc, ExitStack() as ctx:
        cpool = ctx.enter_context(tc.tile_pool(name="cpool", bufs=1))
        xpool = ctx.enter_context(tc.tile_pool(name="xpool", bufs=2))
        hpool = ctx.enter_context(tc.tile_pool(name="hpool", bufs=1))
        wpool = ctx.enter_context(tc.tile_pool(name="wpool", bufs=2))
        epool = ctx.enter_context(tc.tile_pool(name="epool", bufs=2))
        rpool = ctx.enter_context(tc.tile_pool(name="rpool", bufs=2))
        bpool = ctx.enter_context(tc.tile_pool(name="bpool", bufs=3))
        ppg = ctx.enter_context(tc.tile_pool(name="ppg", bufs=2, space="PSUM"))
        ppu = ctx.enter_context(tc.tile_pool(name="ppu", bufs=2, space="PSUM"))
        ppy = ctx.enter_context(tc.tile_pool(name="ppy", bufs=2, space="PSUM"))
        ppys = ctx.enter_context(tc.tile_pool(name="ppys", bufs=1, space="PSUM"))
        ppl = ctx.enter_context(tc.tile_pool(name="ppl", bufs=1, space="PSUM"))
        dpool = ctx.enter_context(tc.tile_pool(name="dpool", bufs=2, space="DRAM"))

        # Residents: router weights, expert one-hot mask, shared-expert weights.
        rt_sb = cpool.tile([128, KH, E], F32)
        nc.sync.dma_start(rt_sb[:], rt)
        em_sb = cpool.tile([128, E], F32)
        nc.sync.dma_start(em_sb[:], em)

        def router(xc):
            """Top-2 router for one chunk -> [128, TC] broadcast of this
            expert's per-token combine weight."""
            pl = ppl.tile([128, NT4, E], F32, name="pl")
            for t4 in range(NT4):
                for k in range(KH):
                    nc.tensor.matmul(
                        pl[:, t4, :],
                        xc[:, k, bass.ts(t4, 128)].bitcast(F32),
                        rt_sb[:, k, :],
                        start=(k == 0),
                        stop=(k == KH - 1),
                    )
            m1 = rpool.tile([128, NT4], F32, name="m1")
            nc.vector.reduce_max(m1[:], pl[:], axis=AX.X)
            eq1 = rpool.tile([128, NT4, E], F32, name="eq1")
            nc.vector.tensor_tensor(
                eq1[:], pl[:], m1[:, :, None].broadcast_to([128, NT4, E]),
                op=ALU.is_equal,
            )
            msk = rpool.tile([128, NT4, E], F32, name="msk")
            nc.vector.scalar_tensor_tensor(
                msk[:], eq1[:], -1e30, pl[:], op0=ALU.mult, op1=ALU.add
            )
            m2 = rpool.tile([128, NT4], F32, name="m2")
            nc.vector.reduce_max(m2[:], msk[:], axis=AX.X)
            dd = rpool.tile([128, NT4], F32, name="dd")
            nc.vector.tensor_tensor(dd[:], m2[:], m1[:], op=ALU.subtract)
            w2 = rpool.tile([128, NT4], F32, name="w2")
            nc.scalar.activation(w2[:], dd[:], AF.Sigmoid)
            eq2 = rpool.tile([128, NT4, E], F32, name="eq2")
            nc.vector.tensor_tensor(
                eq2[:], msk[:], m2[:, :, None].broadcast_to([128, NT4, E]),
                op=ALU.is_equal,
            )
            emb = em_sb[:, None, :].broadcast_to([128, NT4, E])
            t1 = rpool.tile([128, NT4, E], F32, name="t1")
            nc.vector.tensor_tensor(t1[:], eq1[:], emb, op=ALU.mult)
            s1 = rpool.tile([128, NT4], F32, name="s1")
            nc.vector.reduce_sum(s1[:], t1[:], axis=AX.X)
            t2 = rpool.tile([128, NT4, E], F32, name="t2")
            nc.vector.tensor_tensor(t2[:], eq2[:], emb, op=ALU.mult)
            s2 = rpool.tile([128, NT4], F32, name="s2")
            nc.vector.reduce_sum(s2[:], t2[:], axis=AX.X)
            # cw = s1*(1-w2) + s2*w2 = s1 + (s2-s1)*w2
            ds = rpool.tile([128, NT4], F32, name="ds")
            nc.vector.tensor_tensor(ds[:], s2[:], s1[:], op=ALU.subtract)
            dw = rpool.tile([128, NT4], F32, name="dw")
            nc.vector.tensor_tensor(dw[:], ds[:], w2[:], op=ALU.mult)
            cw = rpool.tile([128, NT4], F32, name="cw")
            nc.vector.tensor_tensor(cw[:], dw[:], s1[:], op=ALU.add)
            # Bounce cw through DRAM to get a [128, TC] all-partition
            # broadcast of the chunk's per-token combine weights.
            cwd = dpool.tile([NT4, 128], F32, name="cwd")
            nc.sync.dma_start(cwd.rearrange("a p -> p a"), cw[:])
            cwb = bpool.tile([128, TC], F32, name="cwb")
            nc.sync.dma_start(cwb[:], cwd.flatten().partition_broadcast(128))
            return cwb

        # Chunk pairs: each streamed weight tile is loaded once per pair and
        # used for both 512-token chunks, halving weight DMA traffic.
        for pr in range(NCH // 2):
            xcs, cwbs, h1s, hss = [], [], [], []
            for half in range(2):
                n = 2 * pr + half
                tok = slice(n * TC, (n + 1) * TC)
                xc = xpool.tile([128, KH, TC], F32R, name="xc")
                nc.sync.dma_start(xc[:], xw[:, :, tok])
                xcs.append(xc)
            for half in range(2):
                cwbs.append(router(xcs[half]))
                h1s.append(hpool.tile([128, KI, TC], F32R, name=f"h1{half}",
                                      tag=f"h1{half}"))
                hss.append(hpool.tile([128, KS, TC], F32R, name=f"hs{half}",
                                      tag=f"hs{half}"))

            # ---- Expert gate/up -> h1 = silu(x@Wg) * (x@Wu), [I, Tc] layout.
            for i in range(KI):
                wgt = wpool.tile([128, KH, 128], F32R, name="wgt")
                nc.sync.dma_start(wgt[:], wg[:, i, :, :])
                wut = wpool.tile([128, KH, 128], F32R, name="wut")
                nc.sync.dma_start(wut[:], wu[:, i, :, :])
                for half in range(2):
                    xc, h1 = xcs[half], h1s[half]
                    pg = ppg.tile([128, TC], F32, name="pg")
                    pu = ppu.tile([128, TC], F32, name="pu")
                    for k in range(KH):
                        nc.tensor.matmul(pg[:], wgt[:, k, :], xc[:, k, :],
                                         start=(k == 0), stop=(k == KH - 1))
                    for k in range(KH):
                        nc.tensor.matmul(pu[:], wut[:, k, :], xc[:, k, :],
                                         start=(k == 0), stop=(k == KH - 1))
                    gt = epool.tile([128, TC], F32R, name="gt")
                    nc.scalar.activation(gt[:], pg[:], AF.Silu)
                    nc.vector.tensor_tensor(h1[:, i, :], gt[:], pu[:],
                                            op=ALU.mult)

            # ---- Shared-expert gate/up -> hs, [ISH, Tc] layout.
            for j in range(KS):
                sgt = wpool.tile([128, KH, 128], F32R, name="sgt")
                nc.sync.dma_start(sgt[:], sg[:, j, :, :])
                sut = wpool.tile([128, KH, 128], F32R, name="sut")
                nc.sync.dma_start(sut[:], su[:, j, :, :])
                for half in range(2):
                    xc, hs = xcs[half], hss[half]
                    ps1 = ppg.tile([128, TC], F32, name="pg")
                    ps2 = ppu.tile([128, TC], F32, name="pu")
                    for k in range(KH):
                        nc.tensor.matmul(ps1[:], sgt[:, k, :], xc[:, k, :],
                                         start=(k == 0), stop=(k == KH - 1))
                    for k in range(KH):
                        nc.tensor.matmul(ps2[:], sut[:, k, :], xc[:, k, :],
                                         start=(k == 0), stop=(k == KH - 1))
                    gs = epool.tile([128, TC], F32R, name="gt")
                    nc.scalar.activation(gs[:], ps1[:], AF.Silu)
                    nc.vector.tensor_tensor(hs[:, j, :], gs[:], ps2[:],
                                            op=ALU.mult)

            # ---- Down projections (transposed output) + combine + partial out.
            parts = [dpool.tile([H, TC], F32, name=f"part{half}",
                                tag=f"part{half}") for half in range(2)]

            def half_rs(hh):
                # ReduceScatter rows [hh*512, hh*512+512) of both chunks'
                # partials as soon as they are ready; each core keeps 64 rows.
                for half in range(2):
                    n = 2 * pr + half
                    rs = dpool.tile([64, TC], F32, name="rs")
                    nc.gpsimd.collective_compute(
                        "ReduceScatter",
                        ALU.add,
                        replica_groups=[list(range(NCORES))],
                        ins=[parts[half][hh * 512:(hh + 1) * 512, :]],
                        outs=[rs[:]],
                    )
                    nc.sync.dma_start(yout[n, hh], rs[:])

            for h in range(HT):
                wdt = wpool.tile([128, KI, 128], F32R, name="wdt")
                nc.sync.dma_start(wdt[:], wd[:, h, :, :])
                sdt = wpool.tile([128, KS, 128], F32R, name="sdt")
                nc.sync.dma_start(sdt[:], sd[:, h, :, :])
                for half in range(2):
                    h1, hs, cwb = h1s[half], hss[half], cwbs[half]
                    py = ppy.tile([128, TC], F32, name="py")
                    for k in range(KI):
                        nc.tensor.matmul(py[:], wdt[:, k, :], h1[:, k, :],
                                         start=(k == 0), stop=(k == KI - 1))
                    pys = ppys.tile([128, TC], F32, name="pys")
                    for k in range(KS):
                        nc.tensor.matmul(pys[:], sdt[:, k, :], hs[:, k, :],
                                         start=(k == 0), stop=(k == KS - 1))
                    ot = epool.tile([128, TC], F32, name="ot")
                    nc.vector.tensor_tensor(ot[:], py[:], cwb[:], op=ALU.mult)
                    ot2 = epool.tile([128, TC], F32, name="ot2")
                    nc.vector.tensor_tensor(ot2[:], ot[:], pys[:], op=ALU.add)
                    nc.sync.dma_start(parts[half][bass.ts(h, 128), :], ot2[:])
                if h == HT // 2 - 1:
                    half_rs(0)
            half_rs(1)


_COMPILED = []


def _get_compiled():
    if not _COMPILED:
        nc = bacc.Bacc(
            "TRN2",
            target_bir_lowering=False,
            debug=False,
            enable_asserts=False,
            num_devices=NCORES,
        )
        _build(nc)
        nc.compile()
        _COMPILED.append(nc)
    return _COMPILED[0]


def _prep_inputs(hidden_states, router_w, Wg, Wu, Wd, Sg, Su, Sd):
    """Host-side sharding/layout prep. Returns per-core input maps."""
    f = np.float32
    x = np.ascontiguousarray(np.asarray(hidden_states, dtype=f).reshape(T, H))
    # [p, k, t] with H-index = k*128 + p
    xw = np.ascontiguousarray(x.reshape(T, KH, 128).transpose(2, 1, 0))
    rt = np.ascontiguousarray(
        np.asarray(router_w, dtype=f).T.reshape(KH, 128, E).transpose(1, 0, 2)
    )
    Wg = np.asarray(Wg, dtype=f)
    Wu = np.asarray(Wu, dtype=f)
    Wd = np.asarray(Wd, dtype=f)
    Sg = np.asarray(Sg, dtype=f)
    Su = np.asarray(Su, dtype=f)
    Sd = np.asarray(Sd, dtype=f)

    in_maps = []
    for c in range(NCORES):
        # [p, i, k, col]: Wg[c][k*128+p, i*128+col]
        wgp = np.ascontiguousarray(
            Wg[c].reshape(KH, 128, KI, 128).transpose(1, 2, 0, 3)
        )
        wup = np.ascontiguousarray(
            Wu[c].reshape(KH, 128, KI, 128).transpose(1, 2, 0, 3)
        )
        # [p, h, k, col]: Wd[c][k*128+p, h*128+col]
        wdp = np.ascontiguousarray(
            Wd[c].reshape(KI, 128, HT, 128).transpose(1, 2, 0, 3)
        )
        sgc = Sg[:, c * ISH:(c + 1) * ISH]
        suc = Su[:, c * ISH:(c + 1) * ISH]
        sdc = Sd[c * ISH:(c + 1) * ISH, :]
        sgp = np.ascontiguousarray(
            sgc.reshape(KH, 128, KS, 128).transpose(1, 2, 0, 3)
        )
        sup = np.ascontiguousarray(
            suc.reshape(KH, 128, KS, 128).transpose(1, 2, 0, 3)
        )
        sdp = np.ascontiguousarray(
            sdc.reshape(KS, 128, HT, 128).transpose(1, 2, 0, 3)
        )
        emc = np.zeros((128, E), dtype=f)
        emc[:, c] = 1.0
        in_maps.append(
            {"xw": xw, "rt": rt, "em": emc, "wg": wgp, "wu": wup,
             "wd": wdp, "sg": sgp, "su": sup, "sd": sdp}
        )
    return in_maps


def _run(in_maps, **spmd_kwargs):
    nc = _get_compiled()
    return run_bass_kernel_spmd(nc, in_maps, list(range(NCORES)), **spmd_kwargs)


def _assemble(results):
    """Stitch per-core/per-chunk H-slices into [B, S, H] output."""
    yt = np.empty((H, T), dtype=np.float32)
    for c in range(NCORES):
        yc = results[c]["y"]  # [NCH, 2, 64, TC]
        for n in range(NCH):
            for a in range(2):
                r0 = a * 512 + c * 64
                yt[r0:r0 + 64, n * TC:(n + 1) * TC] = yc[n, a]
    return np.ascontiguousarray(yt.T).reshape(B, S, H)


def kernel(hidden_states, router_w, Wg, Wu, Wd, Sg, Su, Sd):
    in_maps = _prep_inputs(hidden_states, router_w, Wg, Wu, Wd, Sg, Su, Sd)
    res = _run(in_maps)
    out = _assemble(res.results)
    router_z_loss = np.float32(0.0)
    return out, router_z_loss


# revision 11
# speedup vs baseline: 1.0413x; 1.0413x over previous
"""Trainium2 Bass kernel for the MoE problem (top-2-of-8 routing + shared expert).

Strategy (expert-parallel over 8 NeuronCores):
  - Core c owns expert c (dense over all T=4096 tokens: with E=8, K=2 the
    dense per-expert compute spread over 8 cores equals the total routed
    FLOPs, with zero load imbalance and no token dispatch) and a 1/8 slice
    of the shared expert's intermediate dimension.
  - The router (logits -> top-2 -> softmax combine weights) is computed
    replicated on every core in exact fp32; each core extracts its expert's
    per-token combine weight with a one-hot mask input.
  - Main GEMMs run in float32r (reduced-precision fp32 matmul mode, ~4x the
    fp32 rate on the PE array).
  - Per 512-token chunk, each core produces its partial output in
    transposed [H, Tc] layout (expert output scaled by combine weight plus
    shared-expert partial); a ReduceScatter over the 8 cores sums the
    partials, leaving each core with a 128-row H-slice. The host stitches
    the slices and transposes back.

kernel(**inputs) takes the full unsharded inputs and returns the full
output, matching reference.reference()'s (out, router_z_loss) structure.
"""

import numpy as np
from contextlib import ExitStack

import concourse.bass as bass
import concourse.mybir as mybir
import concourse.tile as tile
from concourse import bacc
from concourse.bass_utils import run_bass_kernel_spmd

F32 = mybir.dt.float32
F32R = mybir.dt.float32r
AF = mybir.ActivationFunctionType
ALU = mybir.AluOpType
AX = mybir.AxisListType

# Problem shape (hardcoded per the harness contract).
B, S, H, E, I, IS = 2, 2048, 1024, 8, 2048, 4096
T = B * S                  # 4096 tokens
NCORES = 8
ISH = IS // NCORES         # 512 shared-intermediate per core
TC = 512                   # token chunk
NCH = T // TC              # 8 chunks
KH = H // 128              # 8
KI = I // 128              # 16
KS = ISH // 128            # 4
HT = H // 128              # 8 output H tiles
NT4 = TC // 128            # 4 token tiles per chunk


def _build(nc):
    xw = nc.dram_tensor("xw", [128, KH, T], F32R, kind="ExternalInput").ap()
    rt = nc.dram_tensor("rt", [128, KH, E], F32, kind="ExternalInput").ap()
    em = nc.dram_tensor("em", [128, E], F32, kind="ExternalInput").ap()
    wg = nc.dram_tensor("wg", [128, KI, KH, 128], F32R, kind="ExternalInput").ap()
    wu = nc.dram_tensor("wu", [128, KI, KH, 128], F32R, kind="ExternalInput").ap()
    wd = nc.dram_tensor("wd", [128, HT, KI, 128], F32R, kind="ExternalInput").ap()
    sg = nc.dram_tensor("sg", [128, KS, KH, 128], F32R, kind="ExternalInput").ap()
    su = nc.dram_tensor("su", [128, KS, KH, 128], F32R, kind="ExternalInput").ap()
    sd = nc.dram_tensor("sd", [128, HT, KS, 128], F32R, kind="ExternalInput").ap()
    yout = nc.dram_tensor("y", [NCH, 128, TC], F32, kind="ExternalOutput").ap()

    with tile.TileContext(nc) as tc, ExitStack() as ctx:
        cpool = ctx.enter_context(tc.tile_pool(name="cpool", bufs=1))
        xpool = ctx.enter_context(tc.tile_pool(name="xpool", bufs=2))
        hpool = ctx.enter_context(tc.tile_pool(name="hpool", bufs=1))
        wpool = ctx.enter_context(tc.tile_pool(name="wpool", bufs=2))
        epool = ctx.enter_context(tc.tile_pool(name="epool", bufs=3))
        rpool = ctx.enter_context(tc.tile_pool(name="rpool", bufs=2))
        bpool = ctx.enter_context(tc.tile_pool(name="bpool", bufs=3))
        ppg = ctx.enter_context(tc.tile_pool(name="ppg", bufs=2, space="PSUM"))
        ppu = ctx.enter_context(tc.tile_pool(name="ppu", bufs=2, space="PSUM"))
        ppy = ctx.enter_context(tc.tile_pool(name="ppy", bufs=2, space="PSUM"))
        ppys = ctx.enter_context(tc.tile_pool(name="ppys", bufs=1, space="PSUM"))
        ppl = ctx.enter_context(tc.tile_pool(name="ppl", bufs=1, space="PSUM"))
        dpool = ctx.enter_context(tc.tile_pool(name="dpool", bufs=2, space="DRAM"))

        # Residents: router weights, expert one-hot mask, shared-expert weights.
        rt_sb = cpool.tile([128, KH, E], F32)
        nc.sync.dma_start(rt_sb[:], rt)
        em_sb = cpool.tile([128, E], F32)
        nc.sync.dma_start(em_sb[:], em)

        def router(xc):
            """Top-2 router for one chunk -> [128, TC] broadcast of this
            expert's per-token combine weight."""
            pl = ppl.tile([128, NT4, E], F32, name="pl")
            for t4 in range(NT4):
                for k in range(KH):
                    nc.tensor.matmul(
                        pl[:, t4, :],
                        xc[:, k, bass.ts(t4, 128)].bitcast(F32),
                        rt_sb[:, k, :],
                        start=(k == 0),
                        stop=(k == KH - 1),
                    )
            m1 = rpool.tile([128, NT4], F32, name="m1")
            nc.vector.reduce_max(m1[:], pl[:], axis=AX.X)
            eq1 = rpool.tile([128, NT4, E], F32, name="eq1")
            nc.vector.tensor_tensor(
                eq1[:], pl[:], m1[:, :, None].broadcast_to([128, NT4, E]),
                op=ALU.is_equal,
            )
            msk = rpool.tile([128, NT4, E], F32, name="msk")
            nc.vector.scalar_tensor_tensor(
                msk[:], eq1[:], -1e30, pl[:], op0=ALU.mult, op1=ALU.add
            )
            m2 = rpool.tile([128, NT4], F32, name="m2")
            nc.vector.reduce_max(m2[:], msk[:], axis=AX.X)
            dd = rpool.tile([128, NT4], F32, name="dd")
            nc.vector.tensor_tensor(dd[:], m2[:], m1[:], op=ALU.subtract)
            w2 = rpool.tile([128, NT4], F32, name="w2")
            nc.scalar.activation(w2[:], dd[:], AF.Sigmoid)
            eq2 = rpool.tile([128, NT4, E], F32, name="eq2")
            nc.vector.tensor_tensor(
                eq2[:], msk[:], m2[:, :, None].broadcast_to([128, NT4, E]),
                op=ALU.is_equal,
            )
            emb = em_sb[:, None, :].broadcast_to([128, NT4, E])
            t1 = rpool.tile([128, NT4, E], F32, name="t1")
            nc.vector.tensor_tensor(t1[:], eq1[:], emb, op=ALU.mult)
            s1 = rpool.tile([128, NT4], F32, name="s1")
            nc.vector.reduce_sum(s1[:], t1[:], axis=AX.X)
            t2 = rpool.tile([128, NT4, E], F32, name="t2")
            nc.vector.tensor_tensor(t2[:], eq2[:], emb, op=ALU.mult)
            s2 = rpool.tile([128, NT4], F32, name="s2")
            nc.vector.reduce_sum(s2[:], t2[:], axis=AX.X)
            # cw = s1*(1-w2) + s2*w2 = s1 + (s2-s1)*w2
            ds = rpool.tile([128, NT4], F32, name="ds")
            nc.vector.tensor_tensor(ds[:], s2[:], s1[:], op=ALU.subtract)
            dw = rpool.tile([128, NT4], F32, name="dw")
            nc.vector.tensor_tensor(dw[:], ds[:], w2[:], op=ALU.mult)
            cw = rpool.tile([128, NT4], F32, name="cw")
            nc.vector.tensor_tensor(cw[:], dw[:], s1[:], op=ALU.add)
            # Bounce cw through DRAM to get a [128, TC] all-partition
            # broadcast of the chunk's per-token combine weights.
            cwd = dpool.tile([NT4, 128], F32, name="cwd")
            nc.sync.dma_start(cwd.rearrange("a p -> p a"), cw[:])
            cwb = bpool.tile([128, TC], F32, name="cwb")
            nc.sync.dma_start(cwb[:], cwd.flatten().partition_broadcast(128))
            return cwb

        # Chunk pairs: each streamed weight tile is loaded once per pair and
        # used for both 512-token chunks, halving weight DMA traffic.
        for pr in range(NCH // 2):
            xcs, cwbs, h1s, hss = [], [], [], []
            for half in range(2):
                n = 2 * pr + half
                tok = slice(n * TC, (n + 1) * TC)
                xc = xpool.tile([128, KH, TC], F32R, name="xc")
                nc.sync.dma_start(xc[:], xw[:, :, tok])
                xcs.append(xc)
            for half in range(2):
                cwbs.append(router(xcs[half]))
                h1s.append(hpool.tile([128, KI, TC], F32R, name=f"h1{half}",
                                      tag=f"h1{half}"))
                hss.append(hpool.tile([128, KS, TC], F32R, name=f"hs{half}",
                                      tag=f"hs{half}"))

            # ---- Expert gate/up -> h1 = silu(x@Wg) * (x@Wu), [I, Tc] layout.
            for i in range(KI):
                wgt = wpool.tile([128, KH, 128], F32R, name="wgt")
                nc.sync.dma_start(wgt[:], wg[:, i, :, :])
                wut = wpool.tile([128, KH, 128], F32R, name="wut")
                nc.sync.dma_start(wut[:], wu[:, i, :, :])
                for half in range(2):
                    xc, h1 = xcs[half], h1s[half]
                    pg = ppg.tile([128, TC], F32, name="pg")
                    pu = ppu.tile([128, TC], F32, name="pu")
                    for k in range(KH):
                        nc.tensor.matmul(pg[:], wgt[:, k, :], xc[:, k, :],
                                         start=(k == 0), stop=(k == KH - 1))
                    for k in range(KH):
                        nc.tensor.matmul(pu[:], wut[:, k, :], xc[:, k, :],
                                         start=(k == 0), stop=(k == KH - 1))
                    gt = epool.tile([128, TC], F32R, name="gt")
                    nc.scalar.activation(gt[:], pg[:], AF.Silu)
                    nc.vector.tensor_tensor(h1[:, i, :], gt[:], pu[:],
                                            op=ALU.mult)

            # ---- Shared-expert gate/up -> hs, [ISH, Tc] layout.
            for j in range(KS):
                sgt = wpool.tile([128, KH, 128], F32R, name="sgt")
                nc.sync.dma_start(sgt[:], sg[:, j, :, :])
                sut = wpool.tile([128, KH, 128], F32R, name="sut")
                nc.sync.dma_start(sut[:], su[:, j, :, :])
                for half in range(2):
                    xc, hs = xcs[half], hss[half]
                    ps1 = ppg.tile([128, TC], F32, name="pg")
                    ps2 = ppu.tile([128, TC], F32, name="pu")
                    for k in range(KH):
                        nc.tensor.matmul(ps1[:], sgt[:, k, :], xc[:, k, :],
                                         start=(k == 0), stop=(k == KH - 1))
                    for k in range(KH):
                        nc.tensor.matmul(ps2[:], sut[:, k, :], xc[:, k, :],
                                         start=(k == 0), stop=(k == KH - 1))
                    gs = epool.tile([128, TC], F32R, name="gt")
                    nc.scalar.activation(gs[:], ps1[:], AF.Silu)
                    nc.vector.tensor_tensor(hs[:, j, :], gs[:], ps2[:],
                                            op=ALU.mult)

            # ---- Down projections (transposed output) + combine + partial out.
            parts = [dpool.tile([H, TC], F32, name=f"part{half}",
                                tag=f"part{half}") for half in range(2)]
            for h in range(HT):
                wdt = wpool.tile([128, KI, 128], F32R, name="wdt")
                nc.sync.dma_start(wdt[:], wd[:, h, :, :])
                sdt = wpool.tile([128, KS, 128], F32R, name="sdt")
                nc.sync.dma_start(sdt[:], sd[:, h, :, :])
                for half in range(2):
                    h1, hs, cwb = h1s[half], hss[half], cwbs[half]
                    py = ppy.tile([128, TC], F32, name="py")
                    for k in range(KI):
                        nc.tensor.matmul(py[:], wdt[:, k, :], h1[:, k, :],
                                         start=(k == 0), stop=(k == KI - 1))
                    pys = ppys.tile([128, TC], F32, name="pys")
                    for k in range(KS):
                        nc.tensor.matmul(pys[:], sdt[:, k, :], hs[:, k, :],
                                         start=(k == 0), stop=(k == KS - 1))
                    ot = epool.tile([128, TC], F32, name="ot")
                    nc.vector.tensor_tensor(ot[:], py[:], cwb[:], op=ALU.mult)
                    ot2 = epool.tile([128, TC], F32, name="ot2")
                    nc.vector.tensor_tensor(ot2[:], ot[:], pys[:], op=ALU.add)
                    nc.sync.dma_start(parts[half][bass.ts(h, 128), :], ot2[:])

            # Per-chunk ReduceScatter: core c keeps H rows [128c, 128c+128).
            for half in range(2):
                n = 2 * pr + half
                rs = dpool.tile([128, TC], F32, name="rs")
                nc.gpsimd.collective_compute(
                    "ReduceScatter",
                    ALU.add,
                    replica_groups=[list(range(NCORES))],
                    ins=[parts[half][:]],
                    outs=[rs[:]],
                )
                nc.sync.dma_start(yout[n], rs[:])


_COMPILED = []


def _get_compiled():
    if not _COMPILED:
        nc = bacc.Bacc(
            "TRN2",
            target_bir_lowering=False,
            debug=False,
            enable_asserts=False,
            num_devices=NCORES,
        )
        _build(nc)
        nc.compile()
        _COMPILED.append(nc)
    return _COMPILED[0]


def _prep_inputs(hidden_states, router_w, Wg, Wu, Wd, Sg, Su, Sd):
    """Host-side sharding/layout prep. Returns per-core input maps."""
    f = np.float32
    x = np.ascontiguousarray(np.asarray(hidden_states, dtype=f).reshape(T, H))
    # [p, k, t] with H-index = k*128 + p
    xw = np.ascontiguousarray(x.reshape(T, KH, 128).transpose(2, 1, 0))
    rt = np.ascontiguousarray(
        np.asarray(router_w, dtype=f).T.reshape(KH, 128, E).transpose(1, 0, 2)
    )
    Wg = np.asarray(Wg, dtype=f)
    Wu = np.asarray(Wu, dtype=f)
    Wd = np.asarray(Wd, dtype=f)
    Sg = np.asarray(Sg, dtype=f)
    Su = np.asarray(Su, dtype=f)
    Sd = np.asarray(Sd, dtype=f)

    in_maps = []
    for c in range(NCORES):
        # [p, i, k, col]: Wg[c][k*128+p, i*128+col]
        wgp = np.ascontiguousarray(
            Wg[c].reshape(KH, 128, KI, 128).transpose(1, 2, 0, 3)
        )
        wup = np.ascontiguousarray(
            Wu[c].reshape(KH, 128, KI, 128).transpose(1, 2, 0, 3)
        )
        # [p, h, k, col]: Wd[c][k*128+p, h*128+col]
        wdp = np.ascontiguousarray(
            Wd[c].reshape(KI, 128, HT, 128).transpose(1, 2, 0, 3)
        )
        sgc = Sg[:, c * ISH:(c + 1) * ISH]
        suc = Su[:, c * ISH:(c + 1) * ISH]
        sdc = Sd[c * ISH:(c + 1) * ISH, :]
        sgp = np.ascontiguousarray(
            sgc.reshape(KH, 128, KS, 128).transpose(1, 2, 0, 3)
        )
        sup = np.ascontiguousarray(
            suc.reshape(KH, 128, KS, 128).transpose(1, 2, 0, 3)
        )
        sdp = np.ascontiguousarray(
            sdc.reshape(KS, 128, HT, 128).transpose(1, 2, 0, 3)
        )
        emc = np.zeros((128, E), dtype=f)
        emc[:, c] = 1.0
        in_maps.append(
            {"xw": xw, "rt": rt, "em": emc, "wg": wgp, "wu": wup,
             "wd": wdp, "sg": sgp, "su": sup, "sd": sdp}
        )
    return in_maps


def _run(in_maps, **spmd_kwargs):
    nc = _get_compiled()
    return run_bass_kernel_spmd(nc, in_maps, list(range(NCORES)), **spmd_kwargs)


def _assemble(results):
    """Stitch per-core/per-chunk H-slices into [B, S, H] output."""
    yt = np.empty((H, T), dtype=np.float32)
    for c in range(NCORES):
        yc = results[c]["y"]  # [NCH, 128, TC]
        for n in range(NCH):
            yt[c * 128:(c + 1) * 128, n * TC:(n + 1) * TC] = yc[n]
    return np.ascontiguousarray(yt.T).reshape(B, S, H)


def kernel(hidden_states, router_w, Wg, Wu, Wd, Sg, Su, Sd):
    in_maps = _prep_inputs(hidden_states, router_w, Wg, Wu, Wd, Sg, Su, Sd)
    res = _run(in_maps)
    out = _assemble(res.results)
    router_z_loss = np.float32(0.0)
    return out, router_z_loss
